# revision 7
# baseline (speedup 1.0000x reference)
"""CenterLoss kernel for Trainium2 (8 NeuronCores, data-parallel over batch).

reference: mean(clip(distmat[i, labels[i]])) where
  distmat[i,c] = ||x_i||^2 + ||c_c||^2 - 2 x_i . c_c
i.e. the loss only needs dist_i = ||x_i - centers[labels[i]]||^2 — a gather +
elementwise + reduce; the full (N, C) matmul in the reference is dead work.

Per core (512 rows of the 4096-row batch):
  - labels enter as [128, 4] int32; chunks of columns drive indirect-DMA
    gathers of centers rows (SWDGE, the only indirect path).
  - x chunks enter SBUF via the two HWDGE queues (sync + scalar).
  - The gather uses the SDMA CCE ALU (compute_op=subtract) to write
    centers[label] - x directly into the x tile: no separate subtract op.
  - Per row-column: sum of squares via ScalarE Square+accum or VectorE
    tensor_tensor_reduce, alternated so both engines share the tail.
  - [128, 4] per-row distances DMA out; host concatenates 8 cores, applies the
    clip (a no-op for this data but kept for exactness) and the mean.
"""

import os

import numpy as np

# clears a wedged NeuronCore from a previous crashed run at NRT init
os.environ.setdefault("NEURON_RT_RESET_CORES", "1")

N, D, C = 4096, 512, 10000
NCORES = 8
ROWS_PER_CORE = N // NCORES  # 512
P = 128
J = ROWS_PER_CORE // P  # 4 rows per partition

CLAMP = 1e-12

_cache = {}

# tuning knobs
CHUNKS = [1, 1, 1, 1]  # columns (center rows per partition) per gather chunk
FUSE = True  # subtract during the gather DMA via the CCE ALU
USE_TTR = False  # vector tensor_tensor_reduce for odd columns
SCRATCH_SIZE = 65536  # SWDGE descriptor ring


def _build_nc():
    import concourse.bass as bass
    import concourse.mybir as mybir
    from concourse import bacc
    from concourse.tile import TileContext

    assert sum(CHUNKS) == J

    nc = bacc.Bacc(
        "TRN2",
        target_bir_lowering=False,
        debug=False,
        num_devices=NCORES,
        # default 16KB ring stalls Q7: 512 gather descriptors x 64B = 32KB
        dynamic_dma_scratch_size=SCRATCH_SIZE,
    )
    x = nc.dram_tensor("x", [P, J * D], mybir.dt.float32, kind="ExternalInput")
    labels = nc.dram_tensor("labels", [P, J], mybir.dt.int32, kind="ExternalInput")
    centers = nc.dram_tensor("centers", [C, D], mybir.dt.float32, kind="ExternalInput")
    out = nc.dram_tensor("out", [P, J], mybir.dt.float32, kind="ExternalOutput")

    with TileContext(nc) as tc:
        with (
            tc.tile_pool(name="io", bufs=1) as io_pool,
            tc.tile_pool(name="work", bufs=2) as work,
        ):
            # labels first on sync — the gathers are gated on it
            lab_tile = io_pool.tile([P, J], mybir.dt.int32)
            nc.sync.dma_start(out=lab_tile[:], in_=labels[:])

            # per-chunk x tiles; loads split across the two HWDGE queues
            # (scalar gets chunk 0 so it isn't queued behind the labels DMA)
            xts = []
            hw_engs = [nc.scalar, nc.sync]
            col0 = 0
            for gi, cols in enumerate(CHUNKS):
                xt = io_pool.tile([P, cols * D], mybir.dt.float32)
                xts.append((xt, col0, cols))
                hw_engs[gi % 2].dma_start(
                    out=xt[:], in_=x[:, col0 * D : (col0 + cols) * D]
                )
                col0 += cols

            if FUSE:
                # the CCE ALU only does add/max/min, so the host negates x and
                # the gather accumulates centers[label] + (-x) in place (WAW on
                # xt orders it after the x load); (c-x)^2 == (x-c)^2
                for xt, col0, cols in xts:
                    nc.gpsimd.indirect_dma_start(
                        out=xt[:],
                        out_offset=None,
                        in_=centers[:],
                        in_offset=bass.IndirectOffsetOnAxis(
                            ap=lab_tile[:, col0 : col0 + cols], axis=0
                        ),
                        compute_op=mybir.AluOpType.add,
                    )
            else:
                gts = []
                for xt, col0, cols in xts:
                    gt = io_pool.tile([P, cols * D], mybir.dt.float32)
                    gts.append(gt)
                    nc.gpsimd.indirect_dma_start(
                        out=gt[:],
                        out_offset=None,
                        in_=centers[:],
                        in_offset=bass.IndirectOffsetOnAxis(
                            ap=lab_tile[:, col0 : col0 + cols], axis=0
                        ),
                    )
                for (xt, col0, cols), gt in zip(xts, gts):
                    nc.vector.tensor_tensor(
                        out=xt[:], in0=xt[:], in1=gt[:],
                        op=mybir.AluOpType.subtract,
                    )

            dists = io_pool.tile([P, J], mybir.dt.float32)
            for xt, col0, cols in xts:
                for ci in range(cols):
                    j = col0 + ci
                    sl = xt[:, ci * D : (ci + 1) * D]
                    if not USE_TTR or j % 2 == 0:
                        sq = work.tile([P, D], mybir.dt.float32, tag="sa")
                        nc.scalar.activation(
                            out=sq[:],
                            in_=sl,
                            func=mybir.ActivationFunctionType.Square,
                            accum_out=dists[:, j : j + 1],
                        )
                    else:
                        sq = work.tile([P, D], mybir.dt.float32, tag="sv")
                        nc.vector.tensor_tensor_reduce(
                            out=sq[:],
                            in0=sl,
                            in1=sl,
                            scale=1.0,
                            scalar=0.0,
                            op0=mybir.AluOpType.mult,
                            op1=mybir.AluOpType.add,
                            accum_out=dists[:, j : j + 1],
                        )

            nc.sync.dma_start(out=out[:], in_=dists[:])

    nc.compile()
    return nc


def _run(in_maps, trace=False):
    from concourse.bass_utils import run_bass_kernel_spmd

    if "nc" not in _cache:
        _cache["nc"] = _build_nc()
    return run_bass_kernel_spmd(
        _cache["nc"], in_maps, list(range(NCORES)), trace=trace
    )


def kernel(x, labels, centers, _trace=False):
    x = np.ascontiguousarray(np.asarray(x, dtype=np.float32))
    labels = np.asarray(labels).astype(np.int32)
    centers = np.ascontiguousarray(np.asarray(centers, dtype=np.float32))

    R = ROWS_PER_CORE
    in_maps = []
    for c in range(NCORES):
        lo = c * R
        hi = lo + R
        xs = x[lo:hi].reshape(P, J * D)
        in_maps.append(
            {
                # negated so the gather's CCE add computes centers[label] - x
                "x": np.negative(xs) if FUSE else xs,
                "labels": np.ascontiguousarray(labels[lo:hi].reshape(P, J)),
                "centers": centers,
            }
        )

    res = _run(in_maps, trace=_trace)
    dists = np.concatenate(
        [res.results[c]["out"].reshape(R) for c in range(NCORES)]
    )
    loss = np.clip(dists, CLAMP, 1.0 / CLAMP).mean(dtype=np.float64)
    out = np.asarray(loss, dtype=np.float32)
    if _trace:
        return out, res
    return out


# revision 8
# speedup vs baseline: 2.0920x; 2.0920x over previous
"""CenterLoss kernel for Trainium2 (8 NeuronCores, data-parallel over batch).

reference: mean(clip(distmat[i, labels[i]])) where
  distmat[i,c] = ||x_i||^2 + ||c_c||^2 - 2 x_i . c_c
i.e. the loss only needs dist_i = ||x_i - centers[labels[i]]||^2 — a gather +
elementwise + reduce; the full (N, C) matmul in the reference is dead work.

Per core (512 rows of the 4096-row batch):
  - labels enter as [128, 4] int32; chunks of columns drive indirect-DMA
    gathers of centers rows (SWDGE, the only indirect path).
  - x chunks enter SBUF via the two HWDGE queues (sync + scalar).
  - The gather uses the SDMA CCE ALU (compute_op=subtract) to write
    centers[label] - x directly into the x tile: no separate subtract op.
  - Per row-column: sum of squares via ScalarE Square+accum or VectorE
    tensor_tensor_reduce, alternated so both engines share the tail.
  - [128, 4] per-row distances DMA out; host concatenates 8 cores, applies the
    clip (a no-op for this data but kept for exactness) and the mean.
"""

import os

import numpy as np

# clears a wedged NeuronCore from a previous crashed run at NRT init
os.environ.setdefault("NEURON_RT_RESET_CORES", "1")

N, D, C = 4096, 512, 10000
NCORES = 8
ROWS_PER_CORE = N // NCORES  # 512
P = 128
J = ROWS_PER_CORE // P  # 4 rows per partition

CLAMP = 1e-12

_cache = {}

# tuning knobs
CHUNKS = [1, 1, 1, 1]  # columns (center rows per partition) per gather chunk
FUSE = True  # subtract during the gather DMA via the CCE ALU
USE_TTR = False  # vector tensor_tensor_reduce for odd columns
SCRATCH_SIZE = 65536  # SWDGE descriptor ring


def _build_nc():
    import concourse.bass as bass
    import concourse.mybir as mybir
    from concourse import bacc
    from concourse.tile import TileContext

    assert sum(CHUNKS) == J

    nc = bacc.Bacc(
        "TRN2",
        target_bir_lowering=False,
        debug=False,
        num_devices=NCORES,
        # default 16KB ring stalls Q7: 512 gather descriptors x 64B = 32KB
        dynamic_dma_scratch_size=SCRATCH_SIZE,
    )
    x = nc.dram_tensor("x", [P, J * D], mybir.dt.float32, kind="ExternalInput")
    labels = nc.dram_tensor("labels", [P, J], mybir.dt.int32, kind="ExternalInput")
    centers = nc.dram_tensor("centers", [C, D], mybir.dt.float32, kind="ExternalInput")
    out = nc.dram_tensor("out", [P, J], mybir.dt.float32, kind="ExternalOutput")

    with TileContext(nc) as tc:
        with (
            tc.tile_pool(name="io", bufs=1) as io_pool,
            tc.tile_pool(name="work", bufs=2) as work,
        ):
            # labels first on sync — the gathers are gated on it
            lab_tile = io_pool.tile([P, J], mybir.dt.int32, tag="lab")
            nc.sync.dma_start(out=lab_tile[:], in_=labels[:])

            # per-chunk x tiles; loads split across the two HWDGE queues
            # (scalar gets chunk 0 so it isn't queued behind the labels DMA)
            xts = []
            hw_engs = [nc.scalar, nc.sync]
            col0 = 0
            for gi, cols in enumerate(CHUNKS):
                xt = io_pool.tile([P, cols * D], mybir.dt.float32, tag=f"x{gi}")
                xts.append((xt, col0, cols))
                hw_engs[gi % 2].dma_start(
                    out=xt[:], in_=x[:, col0 * D : (col0 + cols) * D]
                )
                col0 += cols

            if FUSE:
                # the CCE ALU only does add/max/min, so the host negates x and
                # the gather accumulates centers[label] + (-x) in place (WAW on
                # xt orders it after the x load); (c-x)^2 == (x-c)^2
                for xt, col0, cols in xts:
                    nc.gpsimd.indirect_dma_start(
                        out=xt[:],
                        out_offset=None,
                        in_=centers[:],
                        in_offset=bass.IndirectOffsetOnAxis(
                            ap=lab_tile[:, col0 : col0 + cols], axis=0
                        ),
                        compute_op=mybir.AluOpType.add,
                    )
            else:
                gts = []
                for gi, (xt, col0, cols) in enumerate(xts):
                    gt = io_pool.tile([P, cols * D], mybir.dt.float32, tag=f"g{gi}")
                    gts.append(gt)
                    nc.gpsimd.indirect_dma_start(
                        out=gt[:],
                        out_offset=None,
                        in_=centers[:],
                        in_offset=bass.IndirectOffsetOnAxis(
                            ap=lab_tile[:, col0 : col0 + cols], axis=0
                        ),
                    )
                for (xt, col0, cols), gt in zip(xts, gts):
                    nc.vector.tensor_tensor(
                        out=xt[:], in0=xt[:], in1=gt[:],
                        op=mybir.AluOpType.subtract,
                    )

            dists = io_pool.tile([P, J], mybir.dt.float32, tag="dists")
            for xt, col0, cols in xts:
                for ci in range(cols):
                    j = col0 + ci
                    sl = xt[:, ci * D : (ci + 1) * D]
                    if not USE_TTR or j % 2 == 0:
                        sq = work.tile([P, D], mybir.dt.float32, tag="sa")
                        nc.scalar.activation(
                            out=sq[:],
                            in_=sl,
                            func=mybir.ActivationFunctionType.Square,
                            accum_out=dists[:, j : j + 1],
                        )
                    else:
                        sq = work.tile([P, D], mybir.dt.float32, tag="sv")
                        nc.vector.tensor_tensor_reduce(
                            out=sq[:],
                            in0=sl,
                            in1=sl,
                            scale=1.0,
                            scalar=0.0,
                            op0=mybir.AluOpType.mult,
                            op1=mybir.AluOpType.add,
                            accum_out=dists[:, j : j + 1],
                        )

            nc.sync.dma_start(out=out[:], in_=dists[:])

    nc.compile()
    return nc


def _run(in_maps, trace=False):
    from concourse.bass_utils import run_bass_kernel_spmd

    if "nc" not in _cache:
        _cache["nc"] = _build_nc()
    return run_bass_kernel_spmd(
        _cache["nc"], in_maps, list(range(NCORES)), trace=trace
    )


def kernel(x, labels, centers, _trace=False):
    x = np.ascontiguousarray(np.asarray(x, dtype=np.float32))
    labels = np.asarray(labels).astype(np.int32)
    centers = np.ascontiguousarray(np.asarray(centers, dtype=np.float32))

    R = ROWS_PER_CORE
    in_maps = []
    for c in range(NCORES):
        lo = c * R
        hi = lo + R
        xs = x[lo:hi].reshape(P, J * D)
        in_maps.append(
            {
                # negated so the gather's CCE add computes centers[label] - x
                "x": np.negative(xs) if FUSE else xs,
                "labels": np.ascontiguousarray(labels[lo:hi].reshape(P, J)),
                "centers": centers,
            }
        )

    res = _run(in_maps, trace=_trace)
    dists = np.concatenate(
        [res.results[c]["out"].reshape(R) for c in range(NCORES)]
    )
    loss = np.clip(dists, CLAMP, 1.0 / CLAMP).mean(dtype=np.float64)
    out = np.asarray(loss, dtype=np.float32)
    if _trace:
        return out, res
    return out


# revision 9
# speedup vs baseline: 2.5523x; 1.2201x over previous
"""CenterLoss kernel for Trainium2 (8 NeuronCores, data-parallel over batch).

reference: mean(clip(distmat[i, labels[i]])) where
  distmat[i,c] = ||x_i||^2 + ||c_c||^2 - 2 x_i . c_c
i.e. the loss only needs dist_i = ||x_i - centers[labels[i]]||^2 — a gather +
elementwise + reduce; the full (N, C) matmul in the reference is dead work.

Per core (512 rows of the 4096-row batch), using the same expansion as the
reference so no subtract stage is needed:
  - labels enter as [128, 4] int32 and drive per-column indirect-DMA gathers
    of centers rows into per-chunk tiles (SWDGE, the only indirect path).
  - x chunks enter SBUF via the two HWDGE queues (sync + scalar).
  - ScalarE computes sum(x^2) per row early (hidden under the gathers) and
    sum(c^2) per row as each gather lands; VectorE computes sum(x*c) via
    scalar_tensor_tensor's accumulator.  Per-chunk tiles keep the deps
    fine-grained so compute overlaps the remaining gathers.
  - A [128, 3*4] accumulator tile DMAs out; the host combines
    xsq + csq - 2*xc per row, applies the clip (a no-op for this data but
    kept for exactness) and the mean.
"""

import os

import numpy as np

# clears a wedged NeuronCore from a previous crashed run at NRT init
os.environ.setdefault("NEURON_RT_RESET_CORES", "1")

N, D, C = 4096, 512, 10000
NCORES = 8
ROWS_PER_CORE = N // NCORES  # 512
P = 128
J = ROWS_PER_CORE // P  # 4 rows per partition

CLAMP = 1e-12

_cache = {}

# tuning knobs
CHUNKS = [1, 1, 1, 1]  # columns (center rows per partition) per gather chunk
XSQ_ON_VECTOR = ()  # chunk columns whose sum(x^2) runs on VectorE instead
SCRATCH_SIZE = 65536  # SWDGE descriptor ring


def _build_nc():
    import concourse.bass as bass
    import concourse.mybir as mybir
    from concourse import bacc
    from concourse.tile import TileContext

    assert sum(CHUNKS) == J

    nc = bacc.Bacc(
        "TRN2",
        target_bir_lowering=False,
        debug=False,
        num_devices=NCORES,
        # default 16KB ring stalls Q7: 512 gather descriptors x 64B = 32KB
        dynamic_dma_scratch_size=SCRATCH_SIZE,
    )
    x = nc.dram_tensor("x", [P, J * D], mybir.dt.float32, kind="ExternalInput")
    labels = nc.dram_tensor("labels", [P, J], mybir.dt.int32, kind="ExternalInput")
    centers = nc.dram_tensor("centers", [C, D], mybir.dt.float32, kind="ExternalInput")
    # columns: [0, J) = sum(x^2), [J, 2J) = sum(x*c), [2J, 3J) = sum(c^2)
    out = nc.dram_tensor("out", [P, 3 * J], mybir.dt.float32, kind="ExternalOutput")

    with TileContext(nc) as tc:
        with (
            tc.tile_pool(name="io", bufs=1) as io_pool,
            tc.tile_pool(name="work", bufs=2) as work,
        ):
            # labels first on sync — the gathers are gated on it
            lab_tile = io_pool.tile([P, J], mybir.dt.int32, tag="lab")
            nc.sync.dma_start(out=lab_tile[:], in_=labels[:])

            # per-chunk x tiles; loads split across the two HWDGE queues
            # (scalar gets chunk 0 so it isn't queued behind the labels DMA)
            xts = []
            hw_engs = [nc.scalar, nc.sync]
            col0 = 0
            for gi, cols in enumerate(CHUNKS):
                xt = io_pool.tile([P, cols * D], mybir.dt.float32, tag=f"x{gi}")
                xts.append((xt, col0, cols))
                hw_engs[gi % 2].dma_start(
                    out=xt[:], in_=x[:, col0 * D : (col0 + cols) * D]
                )
                col0 += cols

            acc = io_pool.tile([P, 3 * J], mybir.dt.float32, tag="acc")

            # sum(x^2) — runs while the gathers stream in
            for xt, col0, cols in xts:
                for ci in range(cols):
                    j = col0 + ci
                    sl = xt[:, ci * D : (ci + 1) * D]
                    if j in XSQ_ON_VECTOR:
                        sq = work.tile([P, D], mybir.dt.float32, tag="sv")
                        nc.vector.tensor_tensor_reduce(
                            out=sq[:],
                            in0=sl,
                            in1=sl,
                            scale=1.0,
                            scalar=0.0,
                            op0=mybir.AluOpType.mult,
                            op1=mybir.AluOpType.add,
                            accum_out=acc[:, j : j + 1],
                        )
                    else:
                        sq = work.tile([P, D], mybir.dt.float32, tag="sa")
                        nc.scalar.activation(
                            out=sq[:],
                            in_=sl,
                            func=mybir.ActivationFunctionType.Square,
                            accum_out=acc[:, j : j + 1],
                        )

            # per-chunk gathers (plain copies — the CCE-fused variant stalls:
            # its completion sem fires ~3.5us after the DMA vs ~0.35us here)
            gts = []
            for gi, (xt, col0, cols) in enumerate(xts):
                gt = io_pool.tile([P, cols * D], mybir.dt.float32, tag=f"g{gi}")
                gts.append(gt)
                nc.gpsimd.indirect_dma_start(
                    out=gt[:],
                    out_offset=None,
                    in_=centers[:],
                    in_offset=bass.IndirectOffsetOnAxis(
                        ap=lab_tile[:, col0 : col0 + cols], axis=0
                    ),
                )

            # as each gather lands: sum(x*c) on VectorE, sum(c^2) on ScalarE
            for (xt, col0, cols), gt in zip(xts, gts):
                for ci in range(cols):
                    j = col0 + ci
                    xsl = xt[:, ci * D : (ci + 1) * D]
                    gsl = gt[:, ci * D : (ci + 1) * D]
                    xc = work.tile([P, D], mybir.dt.float32, tag="xc")
                    nc.vector.scalar_tensor_tensor(
                        out=xc[:],
                        in0=xsl,
                        scalar=0.0,
                        in1=gsl,
                        op0=mybir.AluOpType.add,
                        op1=mybir.AluOpType.mult,
                        accum_out=acc[:, J + j : J + j + 1],
                    )
                    sq = work.tile([P, D], mybir.dt.float32, tag="sa")
                    nc.scalar.activation(
                        out=sq[:],
                        in_=gsl,
                        func=mybir.ActivationFunctionType.Square,
                        accum_out=acc[:, 2 * J + j : 2 * J + j + 1],
                    )

            nc.sync.dma_start(out=out[:], in_=acc[:])

    nc.compile()
    return nc


def _run(in_maps, trace=False):
    from concourse.bass_utils import run_bass_kernel_spmd

    if "nc" not in _cache:
        _cache["nc"] = _build_nc()
    return run_bass_kernel_spmd(
        _cache["nc"], in_maps, list(range(NCORES)), trace=trace
    )


def kernel(x, labels, centers, _trace=False):
    x = np.ascontiguousarray(np.asarray(x, dtype=np.float32))
    labels = np.asarray(labels).astype(np.int32)
    centers = np.ascontiguousarray(np.asarray(centers, dtype=np.float32))

    R = ROWS_PER_CORE
    in_maps = []
    for c in range(NCORES):
        lo = c * R
        hi = lo + R
        in_maps.append(
            {
                "x": x[lo:hi].reshape(P, J * D),
                "labels": np.ascontiguousarray(labels[lo:hi].reshape(P, J)),
                "centers": centers,
            }
        )

    res = _run(in_maps, trace=_trace)
    parts = []
    for c in range(NCORES):
        a = res.results[c]["out"]  # [P, 3J]
        xsq, xc, csq = a[:, :J], a[:, J : 2 * J], a[:, 2 * J :]
        parts.append((xsq + csq - 2.0 * xc).reshape(R))
    dists = np.concatenate(parts)
    loss = np.clip(dists, CLAMP, 1.0 / CLAMP).mean(dtype=np.float64)
    out = np.asarray(loss, dtype=np.float32)
    if _trace:
        return out, res
    return out
